# revision 9
# baseline (speedup 1.0000x reference)
"""Trainium2 Bass kernel for nn_Decoder_64699387347471.

Attention-LSTM image-caption decoder, data-parallel over batch B=256 on 8
NeuronCores (32 batch elements per core).

Per-core algorithm:
  Phase 1: stream img [32,620,512] f32, cast bf16, PE-transpose, project with
           Ic_w -> attn_img kept resident in SBUF in BOTH layouts:
           AIMT (A-major, for scores) and AIMP (P-major, for ctx), bf16.
  T loop (32 steps): per-step attention (scores -> softmax -> ctx) with
           per-batch matvecs packed via PE column tiling (4 col groups x
           one-hot-column stationaries -> batch-dense PSUM rows), LSTM cell,
           and vocab projection whose transposed output feeds the next step.
"""

import json

import numpy as np
import ml_dtypes

import concourse.bass as bass
import concourse.mybir as mybir
import concourse.tile as tile
from concourse import bass_utils
from concourse.masks import make_identity

# ---------------------------------------------------------------------------
# Workaround for this container's walrus build: any TPB instruction with >1
# semaphore sync-wait fails codegen ("Too many sync wait commands"). Split
# excess waits onto NoOp instructions inserted just before, same engine.
# ---------------------------------------------------------------------------
_orig_to_json_bytes = bass.Bass.to_json_bytes


def _split_waits_json(doc):
    for fn in doc.get("functions", []):
        for blk in fn.get("blocks", []):
            out = []
            for inst in blk.get("instructions", []):
                si = inst.get("sync_info") or {}
                waits = si.get("on_wait") or []
                if len(waits) > 1:
                    for k, w in enumerate(waits[:-1]):
                        nop = {
                            "engine": inst["engine"], "ins": [], "outs": [],
                            "name": f"{inst['name']}w{k}", "opcode": "NoOp",
                            "sync_info": {"on_update": [], "on_wait": [w]},
                        }
                        if "debug" in inst:
                            nop["debug"] = inst["debug"]
                        out.append(nop)
                    si = dict(si)
                    si["on_wait"] = [waits[-1]]
                    inst = dict(inst)
                    inst["sync_info"] = si
                out.append(inst)
            blk["instructions"] = out
    return doc


def _patched_to_json_bytes(self, *args, **kwargs):
    doc = json.loads(_orig_to_json_bytes(self, *args, **kwargs))
    return json.dumps(_split_waits_json(doc)).encode()


bass.Bass.to_json_bytes = _patched_to_json_bytes

# ---------------------------------------------------------------------------

BF = mybir.dt.bfloat16
F32 = mybir.dt.float32
nbf = ml_dtypes.bfloat16

B, P, C, T, V, H, A = 256, 620, 512, 32, 140, 256, 256
NC = 8
BL = B // NC          # 32 batch elements per core
PC = 124              # P-chunk height for P-major layout (5 * 124 = 620)
SCALE = 1.0 / 16.0    # 1/sqrt(A)

AF = mybir.ActivationFunctionType
AX = mybir.AxisListType


def _ap(view, extra_dims, offset_elems=0):
    """Manual AP: keep view's partition dim, replace free dims."""
    return bass.AP(
        tensor=view.tensor,
        offset=view.offset + offset_elems,
        ap=[view.ap[0]] + list(extra_dims),
    )


def build_nc(t_steps=T):
    nc = bass.Bass(dynamic_dma_scratch_size=2048)

    img = nc.dram_tensor("img", [BL, P, C], F32, kind="ExternalInput")
    x0t = nc.dram_tensor("x0t", [V, BL], F32, kind="ExternalInput")
    ic_wt = nc.dram_tensor("ic_wt", [C, A], BF, kind="ExternalInput")
    hc_wt = nc.dram_tensor("hc_wt", [H, A], BF, kind="ExternalInput")
    cd_wt = nc.dram_tensor("cd_wt", [H, V], BF, kind="ExternalInput")
    w7 = nc.dram_tensor("w7", [7 * 128, 4 * H], BF, kind="ExternalInput")
    ic_b2 = nc.dram_tensor("ic_b2", [128, 2], F32, kind="ExternalInput")
    hc_b2 = nc.dram_tensor("hc_b2", [128, 2], F32, kind="ExternalInput")
    cd_b2 = nc.dram_tensor("cd_b2", [128, 2], F32, kind="ExternalInput")
    y = nc.dram_tensor("y", [T, V, BL], F32, kind="ExternalOutput")

    with tile.TileContext(nc) as tc:
        with tc.tile_pool(name="res", bufs=1) as res:
            # resident tensors
            AIMT = res.tile([128, 2, BL, P], BF)       # (a-chunk r, b, p)
            AIMP = res.tile([128, 5, BL, A], BF)       # (p-chunk c, b, a); 124 valid parts
            W7S = res.tile([128, 7, 4 * H], BF)
            ICWT = res.tile([128, 4, A], BF)
            HCWT = res.tile([128, 2, A], BF)
            CDWT = res.tile([128, 2, V], BF)
            ICB = res.tile([128, 2], F32)
            HCB = res.tile([128, 2], F32)
            CDB = res.tile([128, 2], F32)
            IDENT = res.tile([128, 128], BF)
            STATQ = res.tile([128, 2, 8, 4, 32], BF)   # (r, q, j, m)
            STATW = res.tile([128, 5, 8, 4, 32], BF)   # (c, q, j, m); 124 valid parts
            INPT = res.tile([128, 7, BL], BF)          # k-chunks of [x pad ctx h one]
            CST = res.tile([32, H], F32)               # c state, batch-dense rows

            make_identity(nc, IDENT)

            # load weights
            for k in range(4):
                nc.sync.dma_start(out=ICWT[:, k, :], in_=ic_wt[128 * k:128 * (k + 1), :])
            for s in range(2):
                nc.sync.dma_start(out=HCWT[:, s, :], in_=hc_wt[128 * s:128 * (s + 1), :])
                nc.sync.dma_start(out=CDWT[:, s, :], in_=cd_wt[128 * s:128 * (s + 1), :])
            for k in range(7):
                nc.sync.dma_start(out=W7S[:, k, :], in_=w7[128 * k:128 * (k + 1), :])
            nc.sync.dma_start(out=ICB, in_=ic_b2[:, :])
            nc.sync.dma_start(out=HCB, in_=hc_b2[:, :])
            nc.sync.dma_start(out=CDB, in_=cd_b2[:, :])

            nc.vector.memset(STATQ, 0.0)
            nc.vector.memset(STATW, 0.0)
            nc.vector.memset(INPT, 0.0)
            nc.vector.memset(CST, 0.0)
            nc.vector.memset(INPT[0:1, 6, :], 1.0)

            # ---------------- phase 1 ----------------
            with tc.tile_pool(name="p1", bufs=2) as p1, \
                 tc.tile_pool(name="p1ps", bufs=2, space="PSUM") as p1ps, \
                 tc.tile_pool(name="p1ps2", bufs=2, space="PSUM") as p1ps2:
                # x0 -> INPT chunks 0,1 (cast bf16)
                x0f = p1.tile([128, BL], F32, tag="x0")
                x0f2 = p1.tile([12, BL], F32, tag="x0b")
                nc.sync.dma_start(out=x0f, in_=x0t[0:128, :])
                nc.sync.dma_start(out=x0f2, in_=x0t[128:140, :])
                nc.scalar.copy(INPT[:, 0, :], x0f)
                nc.scalar.copy(INPT[0:12, 1, :], x0f2)

                for b in range(BL):
                    imgT = p1.tile([128, 4, P], BF, tag="imgT")
                    for c in range(5):
                        imgf = p1.tile([PC, C], F32, tag="imgf")
                        nc.sync.dma_start(out=imgf, in_=img[b, PC * c:PC * (c + 1), :])
                        imgb = p1.tile([PC, C], BF, tag="imgb")
                        nc.gpsimd.tensor_copy(out=imgb, in_=imgf)
                        for cc in range(4):
                            tp = p1ps2.tile([128, PC], BF, tag="tp")
                            nc.tensor.transpose(tp, imgb[:, 128 * cc:128 * (cc + 1)],
                                                IDENT[0:PC, 0:PC])
                            nc.scalar.copy(imgT[:, cc, PC * c:PC * (c + 1)], tp)
                    # attn_imgT = Ic_wT.T @ imgT  (A-major)
                    for r in range(2):
                        aps = p1ps.tile([128, P], F32, tag="aimt")
                        for k in range(4):
                            for lo, hi in ((0, 512), (512, P)):
                                nc.tensor.matmul(
                                    aps[:, lo:hi],
                                    ICWT[:, k, 128 * r:128 * (r + 1)],
                                    imgT[:, k, lo:hi],
                                    start=(k == 0), stop=(k == 3),
                                    skip_group_check=True)
                        nc.scalar.activation(AIMT[:, r, b, :], aps, AF.Identity,
                                             bias=ICB[:, r:r + 1])
                    # P-major via transpose of AIMT
                    for r in range(2):
                        for c in range(5):
                            pm = p1ps2.tile([PC, 128], BF, tag="pm")
                            nc.tensor.transpose(
                                pm, AIMT[:, r, b, PC * c:PC * (c + 1)], IDENT)
                            nc.vector.tensor_copy(
                                out=AIMP[0:PC, c, b, 128 * r:128 * (r + 1)], in_=pm)

            # ---------------- T loop ----------------
            with tc.tile_pool(name="tl", bufs=2) as tl, \
                 tc.tile_pool(name="tle", bufs=1) as tle, \
                 tc.tile_pool(name="tlps", bufs=1, space="PSUM") as tlps, \
                 tc.tile_pool(name="tlps2", bufs=6, space="PSUM") as tlps2:
                for t in range(t_steps):
                    # ---- q = Hc(h): qT [2x128 a, 32 b] ----
                    qps = tlps2.tile([128, 2, BL], F32, tag="w")
                    for r in range(2):
                        for s in range(2):
                            nc.tensor.matmul(
                                qps[:, r, :],
                                HCWT[:, s, 128 * r:128 * (r + 1)],
                                INPT[:, 4 + s, :],
                                start=(s == 0), stop=(s == 1),
                                skip_group_check=True)
                    # scatter qT into STATQ one-hot cols (+Hc_b, cast bf16)
                    for r in range(2):
                        src = qps[:, r, :].rearrange("p (j q) -> p j q", j=4)
                        dst = _ap(STATQ[:, 0, 0, 0, :], [[32, 4], [129, 8]],
                                  offset_elems=r * 1024)
                        nc.scalar.activation(dst, src, AF.Identity,
                                             bias=HCB[:, r:r + 1])

                    # ---- scores [128, 620]: rows 32j+q = batch 8j+q ----
                    sps = tlps.tile([128, P], F32, tag="big")
                    for q in range(8):
                        for r in range(2):
                            for j in range(4):
                                b = 8 * j + q
                                for lo, hi in ((0, 512), (512, P)):
                                    nc.tensor.matmul(
                                        sps[32 * j:32 * (j + 1), lo:hi],
                                        STATQ[:, r, q, j, :],
                                        AIMT[:, r, b, lo:hi],
                                        start=(q == 0 and r == 0),
                                        stop=(q == 7 and r == 1),
                                        tile_position=(0, 32 * j),
                                        skip_group_check=True)

                    # ---- softmax (deferred normalization) ----
                    mx = tl.tile([128, 1], F32, tag="mx")
                    nc.vector.reduce_max(out=mx, in_=sps, axis=AX.X)
                    nmx = tl.tile([128, 1], F32, tag="nmx")
                    nc.vector.tensor_scalar_mul(nmx, mx, -SCALE)
                    wsb = tl.tile([128, P], BF, tag="wsb")
                    zs = tl.tile([128, 1], F32, tag="zs")
                    nc.scalar.activation(wsb, sps, AF.Exp, bias=nmx, scale=SCALE,
                                         accum_out=zs)
                    zi = tl.tile([128, 1], F32, tag="zi")
                    nc.vector.reciprocal(out=zi, in_=zs)

                    # ---- wT into STATW via PE transpose + scatter ----
                    for c in range(5):
                        wtp = tlps2.tile([PC, 128], BF, tag="w")
                        nc.tensor.transpose(wtp, wsb[:, PC * c:PC * (c + 1)], IDENT)
                        src = wtp.rearrange("p (j m) -> p j m", j=4)[:, :, 0:8]
                        dst = _ap(STATW[0:PC, 0, 0, 0, :], [[32, 4], [129, 8]],
                                  offset_elems=c * 1024)
                        nc.scalar.copy(dst, src)

                    # ---- ctx [128, 256]: rows 32j+q = batch 8j+q ----
                    cps = tlps2.tile([128, A], F32, tag="w")
                    for q in range(8):
                        for c in range(5):
                            for j in range(4):
                                b = 8 * j + q
                                nc.tensor.matmul(
                                    cps[32 * j:32 * (j + 1), :],
                                    STATW[0:PC, c, q, j, :],
                                    AIMP[0:PC, c, b, :],
                                    start=(q == 0 and c == 0),
                                    stop=(q == 7 and c == 4),
                                    tile_position=(0, 32 * j),
                                    skip_group_check=True)
                    # normalize by 1/Z, cast bf16
                    csb = tl.tile([128, A], BF, tag="csb")
                    nc.scalar.mul(csb, cps, zi)
                    # ctxT -> INPT chunks 2,3 (compact scattered cols to dense b)
                    for r in range(2):
                        ctp = tlps2.tile([128, 128], BF, tag="w")
                        nc.tensor.transpose(ctp, csb[:, 128 * r:128 * (r + 1)], IDENT)
                        src = ctp.rearrange("p (j m) -> p j m", j=4)[:, :, 0:8]
                        dst = INPT[:, 2 + r, :].rearrange("p (j q) -> p j q", j=4)
                        nc.scalar.copy(dst, src)

                    # ---- LSTM gates [128, 256]: group j = gate j, batch-dense ----
                    gps = tlps2.tile([128, A], F32, tag="w")
                    for k in range(7):
                        for j in range(4):
                            nc.tensor.matmul(
                                gps[32 * j:32 * (j + 1), :],
                                INPT[:, k, :],
                                W7S[:, k, A * j:A * (j + 1)],
                                start=(k == 0), stop=(k == 6),
                                tile_position=(0, 32 * j),
                                skip_group_check=True)
                    si = tle.tile([32, H], F32, tag="si")
                    sf = tle.tile([32, H], F32, tag="sf")
                    tg = tle.tile([32, H], F32, tag="tg")
                    so = tle.tile([32, H], F32, tag="so")
                    nc.scalar.activation(si, gps[0:32, :], AF.Sigmoid)
                    nc.scalar.activation(sf, gps[32:64, :], AF.Sigmoid)
                    nc.scalar.activation(tg, gps[64:96, :], AF.Tanh)
                    nc.scalar.activation(so, gps[96:128, :], AF.Sigmoid)
                    u = tle.tile([32, H], F32, tag="u")
                    nc.vector.tensor_mul(u, si, tg)
                    v = tle.tile([32, H], F32, tag="v")
                    nc.vector.tensor_mul(v, sf, CST)
                    nc.vector.tensor_add(CST, u, v)
                    tc2 = tle.tile([32, H], F32, tag="tc2")
                    nc.scalar.activation(tc2, CST, AF.Tanh)
                    h2b = tl.tile([32, H], BF, tag="h2b")
                    nc.vector.tensor_mul(h2b, so, tc2)

                    # hT -> INPT chunks 4,5
                    for r in range(2):
                        htp = tlps2.tile([128, 32], BF, tag="w")
                        nc.tensor.transpose(htp, h2b[:, 128 * r:128 * (r + 1)],
                                            IDENT[0:32, 0:32])
                        nc.scalar.copy(INPT[:, 4 + r, :], htp)

                    # ---- logits: lhsT = CDWT, rhs = hT ----
                    lg0 = tlps2.tile([128, BL], F32, tag="w")
                    lg1 = tlps2.tile([12, BL], F32, tag="w")
                    for s in range(2):
                        nc.tensor.matmul(lg0, CDWT[:, s, 0:128], INPT[:, 4 + s, :],
                                         start=(s == 0), stop=(s == 1),
                                         skip_group_check=True)
                        nc.tensor.matmul(lg1, CDWT[:, s, 128:140], INPT[:, 4 + s, :],
                                         start=(s == 0), stop=(s == 1),
                                         skip_group_check=True)
                    ys0 = tl.tile([128, BL], F32, tag="ys0")
                    ys1 = tl.tile([12, BL], F32, tag="ys1")
                    nc.scalar.activation(ys0, lg0, AF.Identity, bias=CDB[:, 0:1])
                    nc.scalar.activation(ys1, lg1, AF.Identity, bias=CDB[0:12, 1:2])
                    nc.gpsimd.dma_start(out=y[t, 0:128, :], in_=ys0)
                    nc.gpsimd.dma_start(out=y[t, 128:140, :], in_=ys1)
                    # feedback x_{t+1} = logits (bf16) -> INPT chunks 0,1
                    nc.scalar.activation(INPT[:, 0, :], lg0, AF.Identity,
                                         bias=CDB[:, 0:1])
                    nc.scalar.activation(INPT[0:12, 1, :], lg1, AF.Identity,
                                         bias=CDB[0:12, 1:2])
    return nc


_NC_CACHE = {}


def _get_nc(t_steps=T):
    if t_steps not in _NC_CACHE:
        _NC_CACHE[t_steps] = build_nc(t_steps)
    return _NC_CACHE[t_steps]


class _Runner:
    """Cached PJRT executable over 8 cores (mirror of bass2jax.run_bass_via_pjrt
    multi-core path, but reusable across calls so compile happens once)."""

    def __init__(self, nc):
        import jax
        from jax.sharding import Mesh, PartitionSpec
        from jax.experimental.shard_map import shard_map
        from concourse import bass2jax, mybir as _mb

        bass2jax.install_neuronx_cc_hook()
        self.nc = nc
        pname = nc.partition_id_tensor.name if nc.partition_id_tensor else None
        in_names, out_names, out_avals, zero_outs = [], [], [], []
        for alloc in nc.m.functions[0].allocations:
            if not isinstance(alloc, _mb.MemoryLocationSet):
                continue
            name = alloc.memorylocations[0].name
            if alloc.kind == "ExternalInput":
                if name != pname:
                    in_names.append(name)
            elif alloc.kind == "ExternalOutput":
                shape = tuple(alloc.tensor_shape)
                dt = _mb.dt.np(alloc.dtype)
                out_names.append(name)
                out_avals.append(jax.core.ShapedArray(shape, dt))
                zero_outs.append(np.zeros(shape, dt))
        self.in_names, self.out_names = in_names, out_names
        self.out_avals, self.zero_outs = out_avals, zero_outs
        n_params = len(in_names)
        all_names = in_names + out_names
        if pname is not None:
            all_names = all_names + [pname]
        donate = tuple(range(n_params, n_params + len(out_names)))

        def _body(*args):
            operands = list(args)
            if pname is not None:
                operands.append(bass2jax.partition_id_tensor())
            outs = bass2jax._bass_exec_p.bind(
                *operands,
                out_avals=tuple(out_avals),
                in_names=tuple(all_names),
                out_names=tuple(out_names),
                lowering_input_output_aliases=(),
                sim_require_finite=True,
                sim_require_nnan=True,
                nc=nc,
            )
            return tuple(outs)

        devices = jax.devices()[:NC]
        self.mesh = Mesh(np.asarray(devices), ("core",))
        self.pspec = PartitionSpec("core")
        in_specs = (self.pspec,) * (n_params + len(out_names))
        out_specs = (self.pspec,) * len(out_names)
        self.fn = jax.jit(
            shard_map(_body, mesh=self.mesh, in_specs=in_specs,
                      out_specs=out_specs, check_rep=False),
            donate_argnums=donate, keep_unused=True)

    def concat_inputs(self, in_maps):
        return [np.concatenate([np.asarray(m[n]) for m in in_maps], axis=0)
                for n in self.in_names]

    def zeros(self):
        return [np.zeros((NC * z.shape[0], *z.shape[1:]), z.dtype)
                for z in self.zero_outs]

    def __call__(self, concat_in, zeros):
        import jax
        outs = self.fn(*concat_in, *zeros)
        jax.block_until_ready(outs)
        return outs

    def split_outputs(self, outs):
        res = []
        for c in range(NC):
            d = {}
            for i, n in enumerate(self.out_names):
                d[n] = np.asarray(outs[i]).reshape(NC, *self.out_avals[i].shape)[c]
            res.append(d)
        return res


_RUNNER = None


def _get_runner():
    global _RUNNER
    if _RUNNER is None:
        _RUNNER = _Runner(_get_nc())
    return _RUNNER


def _prep_host(inputs):
    Ic_w = np.asarray(inputs["Ic_w"], np.float32)
    Hc_w = np.asarray(inputs["Hc_w"], np.float32)
    Cd_w = np.asarray(inputs["Cd_w"], np.float32)
    W_ih = np.asarray(inputs["W_ih"], np.float32)
    W_hh = np.asarray(inputs["W_hh"], np.float32)
    b_cat = np.asarray(inputs["b_ih"], np.float32) + np.asarray(inputs["b_hh"], np.float32)

    w7 = np.zeros((7 * 128, 4 * H), np.float32)
    w7[0:V, :] = W_ih[:, 0:V].T
    w7[256:512, :] = W_ih[:, V:V + A].T
    w7[512:768, :] = W_hh.T
    w7[768, :] = b_cat

    def pack2(bvec, n):
        out = np.zeros((128, 2), np.float32)
        out[:, 0] = bvec[0:128]
        out[0:n - 128, 1] = bvec[128:n]
        return out

    return {
        "ic_wt": Ic_w.T.astype(nbf).copy(),
        "hc_wt": Hc_w.T.astype(nbf).copy(),
        "cd_wt": Cd_w.T.astype(nbf).copy(),
        "w7": w7.astype(nbf),
        "ic_b2": pack2(np.asarray(inputs["Ic_b"], np.float32), A),
        "hc_b2": pack2(np.asarray(inputs["Hc_b"], np.float32), A),
        "cd_b2": pack2(np.asarray(inputs["Cd_b"], np.float32), V),
    }


def kernel(**inputs):
    nc = _get_nc()
    shared = _prep_host(inputs)
    img = np.ascontiguousarray(np.asarray(inputs["img_features"], np.float32))
    targ0 = np.asarray(inputs["targets"], np.float32)[:, 0, :]

    in_maps = []
    for core in range(NC):
        sl = slice(core * BL, (core + 1) * BL)
        m = dict(shared)
        m["img"] = img[sl]
        m["x0t"] = np.ascontiguousarray(targ0[sl].T)
        in_maps.append(m)

    runner = _get_runner()
    outs = runner(runner.concat_inputs(in_maps), runner.zeros())
    results = runner.split_outputs(outs)
    out = np.zeros((B, T, V), np.float32)
    for core in range(NC):
        ys = results[core]["y"]                # [T, V, BL]
        out[core * BL:(core + 1) * BL] = np.transpose(ys, (2, 0, 1))
    return out
